# revision 15
# baseline (speedup 1.0000x reference)
"""Cross-attention (Bahdanau-style) scores kernel for 8 Trainium2 NeuronCores.

Reference computation (per batch b, source position s):
    energy[b,s,:] = tanh(Wh @ h[b] + We @ eo[s,b] + bias)
    scores[b,s]   = v . energy[b,s,:]
    out[b,:]      = softmax(scores[b,:])   over s

Sharding: data-parallel over batch (64 batches -> 8 per core). Weights are
replicated. No collectives needed (softmax is per-batch, fully local).

Per-core structure (S=4096, Bc=8, E2=512, D=256), blocks of 4 batches:
  - DMA pre-transposed eoT tiles [p=128, c=4, s=512] bf16 (4 KiB lines)
  - PE proj: for kc, c: one We chunk stationary, streamed over 4 batch
    tiles (stationary reuse -> LDWEIGHTS hides under matmuls)
  - ACT: energy = tanh(psum + baseT[k,bb]) fused bias, -> bf16 SBUF
  - PE dot: v . energy via col-tiled [128,32] matmuls, 4 concurrent
    quadrants (tile_position=(0,32q)) -> scores land on partition
    32*(bb%4) + bb//4 of a full [128,512] PSUM bank
  - ACT: exp per s-group with accum -> per-group sums
  - tail: DVE reduce + reciprocal + scale, 8 row DMAs out
"""

import numpy as np
import ml_dtypes

import concourse.bass as bass
import concourse.bacc as bacc
import concourse.tile as tile
from concourse import mybir
from concourse.bass_utils import run_bass_kernel_spmd

dt = mybir.dt

S = 4096          # src_len
B = 64            # global batch
E2 = 512          # 2*enc_hid
D = 256           # dec_hid
NCORES = 8
BC = B // NCORES  # batches per core = 8
P = 128
SG = 512          # s-group size
NG = S // SG      # 8 s-groups
NEC = E2 // P     # 4 e-chunks
NKC = D // P      # 2 k-chunks
HB = 4            # batches per block-half
NB = NG * 2       # 16 blocks (s-group x batch-half)

F32 = dt.float32
BF16 = dt.bfloat16


def _p_of(bb):
    """Partition where batch bb's scores land (quadrant q=bb%4, col bb//4)."""
    return 32 * (bb % 4) + bb // 4


def build_program():
    nc = bacc.Bacc(None, target_bir_lowering=False, debug=False, num_devices=8)

    # eoT[bb, p, g, c, j] = eo[g*512+j, bb, c*128+p]  (host pre-transposed, bf16)
    eoT_d = nc.declare_dram_parameter("eoT", [BC, P, NG, NEC, SG], BF16, isOutput=False)
    # weT[p, c, k] = We.T[c*128+p, k] ; We = W[:, D:]
    weT_d = nc.declare_dram_parameter("weT", [P, NEC, D], BF16, isOutput=False)
    # whT[p, dc, kc, j] = W[kc*128+j, dc*128+p]  (Wh part, pre-chunked)
    whT_d = nc.declare_dram_parameter("whT", [P, NKC, NKC, P], F32, isOutput=False)
    # hT[p, dc, bb] = h[bb, dc*128+p]
    hT_d = nc.declare_dram_parameter("hT", [P, NKC, BC], F32, isOutput=False)
    # bT[p, kc] = bias[kc*128+p]
    bT_d = nc.declare_dram_parameter("bT", [P, NKC], F32, isOutput=False)
    # vm[p, kc, bb, col] = v[kc*128+p] if col == bb//4 else 0
    vm_d = nc.declare_dram_parameter("vm", [P, NKC, BC, 32], BF16, isOutput=False)
    out_d = nc.declare_dram_parameter("out", [BC, S], F32, isOutput=True)

    with tile.TileContext(nc) as tc:
        with tc.tile_pool(name="consts", bufs=1) as consts:
            # DMA priority: weT + block-0 tiles feed the first matmuls;
            # everything else can land later
            weT = consts.tile([P, NEC, D], BF16)
            nc.sync.dma_start(out=weT, in_=weT_d[:])
            eot0 = consts.tile([P, HB, NEC, SG], BF16, name="eot0")
            for i in range(HB):
                for c in range(NEC):
                    nc.sync.dma_start(
                        out=eot0[:, i, c, :], in_=eoT_d[i, :, 0, c, :]
                    )
            whT = consts.tile([P, NKC, NKC, P], F32)
            nc.sync.dma_start(out=whT, in_=whT_d[:])
            hT = consts.tile([P, NKC, BC], F32)
            nc.sync.dma_start(out=hT, in_=hT_d[:])
            bT = consts.tile([P, NKC], F32)
            nc.sync.dma_start(out=bT, in_=bT_d[:])
            vm = consts.tile([P, NKC, BC, 32], BF16)
            nc.sync.dma_start(out=vm, in_=vm_d[:])

            baseT = consts.tile([P, NKC, BC], F32)   # [k128, kc, bb]
            esums = consts.tile([P, NG], F32)        # per-group exp sums
            out_sb = consts.tile([P, S], F32)        # exp(scores), batch on partition

            # --- init: baseT[k, bb] = sum_d Wh[k, d] h[bb, d] + bias[k] ---
            with tc.tile_pool(name="initps", bufs=1, space="PSUM") as initps:
                ps_base = initps.tile([P, NKC, BC], F32)
                for kc in range(NKC):
                    for dc in range(NKC):
                        nc.tensor.matmul(
                            ps_base[:, kc, :],
                            whT[:, dc, kc, :],
                            hT[:, dc, :],
                            start=(dc == 0),
                            stop=(dc == NKC - 1),
                        )
                for kc in range(NKC):
                    nc.vector.tensor_scalar_add(
                        baseT[:, kc, :], ps_base[:, kc, :], bT[:, kc : kc + 1]
                    )

            with (
                tc.tile_pool(name="eot", bufs=12) as eot_pool,
                tc.tile_pool(name="en", bufs=16) as en_pool,
                tc.tile_pool(name="pep", bufs=6, space="PSUM") as pep_pool,
                tc.tile_pool(name="psc", bufs=2, space="PSUM") as psc_pool,
            ):
                pend = None  # (g, half, en tiles of previous block)
                ps_sc = None

                def emit_dots(g, half, ens):
                    """Col-tiled dot matmuls for one block-half; 4 concurrent
                    quadrants. Accumulation group per quadrant spans both
                    halves (bb and bb+4) and both kc."""
                    nonlocal ps_sc
                    if half == 0:
                        ps_sc = psc_pool.tile([P, SG], F32, tag="psc")
                    for kc in range(NKC):
                        for i in range(HB):
                            bb = half * HB + i
                            q = bb % 4
                            nc.tensor.matmul(
                                ps_sc[32 * q : 32 * q + 32, :],
                                vm[:, kc, bb, :],
                                ens[(i, kc)],
                                start=(half == 0 and kc == 0),
                                stop=(half == 1 and kc == NKC - 1),
                                tile_position=(0, 32 * q),
                            )
                    if half == 1:
                        s0 = g * SG
                        nc.scalar.activation(
                            out=out_sb[:, s0 : s0 + SG], in_=ps_sc,
                            func=mybir.ActivationFunctionType.Exp,
                            accum_out=esums[:, g : g + 1],
                        )

                for b in range(NB):
                    g, half = b // 2, b % 2
                    # ---- DMA this block's 4 batch tiles (4 KiB lines) ----
                    if b == 0:
                        # block 0 was DMA'd with the consts (high priority)
                        eots = [eot0[:, i] for i in range(HB)]
                    else:
                        eots = []
                        for i in range(HB):
                            bb = half * HB + i
                            t = eot_pool.tile([P, NEC, SG], BF16, tag="eot")
                            nc.sync.dma_start(out=t, in_=eoT_d[bb, :, g])
                            eots.append(t)

                    ens = {}
                    if b == 0:
                        # first block: batch-outer so the first matmul only
                        # waits for tile 0, streaming as the DMAs land
                        for i in range(HB):
                            for kc in range(NKC):
                                ps = pep_pool.tile(
                                    [P, SG], F32, tag="pep", name=f"ps0_{kc}_{i}"
                                )
                                for c in range(NEC):
                                    nc.tensor.matmul(
                                        ps, weT[:, c, kc * P : (kc + 1) * P],
                                        eots[i][:, c, :],
                                        start=(c == 0), stop=(c == NEC - 1),
                                    )
                                en = en_pool.tile([P, SG], BF16, tag="en",
                                                  name=f"en0_{kc}_{i}")
                                nc.scalar.activation(
                                    out=en, in_=ps,
                                    func=mybir.ActivationFunctionType.Tanh,
                                    bias=baseT[:, kc, i : i + 1],
                                )
                                ens[(i, kc)] = en
                        pend = (g, half, ens)
                        continue
                    for kc in range(NKC):
                        # ---- projection, We chunk stationary reused 4x ----
                        pss = [
                            pep_pool.tile([P, SG], F32, tag="pep", name=f"ps_{b}_{kc}_{i}")
                            for i in range(HB)
                        ]
                        for c in range(NEC):
                            lhs = weT[:, c, kc * P : (kc + 1) * P]
                            for i in range(HB):
                                nc.tensor.matmul(
                                    pss[i], lhs, eots[i][:, c, :],
                                    start=(c == 0), stop=(c == NEC - 1),
                                )
                        # ---- tanh(+base bias) -> bf16 energies ----
                        for i in range(HB):
                            bb = half * HB + i
                            en = en_pool.tile([P, SG], BF16, tag="en")
                            nc.scalar.activation(
                                out=en, in_=pss[i],
                                func=mybir.ActivationFunctionType.Tanh,
                                bias=baseT[:, kc, bb : bb + 1],
                            )
                            ens[(i, kc)] = en
                        # previous block's dots go after this block's first
                        # proj wave so their tanhs have finished
                        if kc == 0 and pend is not None:
                            emit_dots(*pend)
                            pend = None
                    pend = (g, half, ens)

                emit_dots(*pend)

                # ---- softmax tail: scale halves on ACT + DVE in parallel,
                # then many small output DMAs to spread across queues ----
                with tc.tile_pool(name="sm", bufs=1) as sm:
                    esum = sm.tile([P, 1], F32)
                    nc.vector.tensor_reduce(
                        out=esum, in_=esums, axis=mybir.AxisListType.X,
                        op=mybir.AluOpType.add,
                    )
                    rsum = sm.tile([P, 1], F32)
                    nc.vector.reciprocal(rsum, esum)
                    # DVE is ~1.6x faster per column than ACT Copy: split 5:3
                    H = 2560
                    nc.vector.tensor_scalar_mul(
                        out_sb[:, :H], out_sb[:, :H], rsum
                    )
                    nc.scalar.activation(
                        out=out_sb[:, H:], in_=out_sb[:, H:],
                        func=mybir.ActivationFunctionType.Copy,
                        scale=rsum,
                    )
                    # scores for batch bb live on partition 32*(bb%4)+bb//4;
                    # stride-32 partition APs: one 4-row DMA per column group
                    osv = out_sb.rearrange("(q r) s -> q r s", q=4)
                    nc.sync.dma_start(out=out_d[0:4, :], in_=osv[:, 0, :])
                    nc.sync.dma_start(out=out_d[4:8, :], in_=osv[:, 1, :])

    return nc


_nc = None


def _get_nc():
    global _nc
    if _nc is None:
        _nc = build_program()
        _nc.compile()
    return _nc


def kernel(hidden, encoder_outputs, W, b, v):
    hidden = np.asarray(hidden, dtype=np.float32)
    encoder_outputs = np.ascontiguousarray(encoder_outputs, dtype=np.float32)
    W = np.asarray(W, dtype=np.float32)
    b = np.asarray(b, dtype=np.float32)
    v = np.asarray(v, dtype=np.float32)

    # host-side prep of the small replicated weights
    We = W[:, D:]                                     # [256, 512]
    weT = np.ascontiguousarray(
        We.T.reshape(NEC, P, D).transpose(1, 0, 2)    # [p, c, k]
    ).astype(ml_dtypes.bfloat16)
    Wh = W[:, :D]                                     # [k, d]
    whT = np.ascontiguousarray(
        Wh.reshape(NKC, P, NKC, P).transpose(3, 2, 0, 1)  # [p(d), dc, kc, j(k)]
    )
    bT = np.ascontiguousarray(b.reshape(NKC, P).T)    # [p, kc]
    vT = np.ascontiguousarray(v.reshape(NKC, P).T)    # [p, kc]
    vm = np.zeros((P, NKC, BC, 32), dtype=np.float32)
    for bb in range(BC):
        vm[:, :, bb, bb // 4] = vT
    vm = vm.astype(ml_dtypes.bfloat16)
    h = hidden[0]                                     # [64, 256]

    nc = _get_nc()
    eo_bf16 = encoder_outputs.astype(ml_dtypes.bfloat16)
    # [S, B, E2] -> [B, E2, S]; per-core slice repacks to [BC, P, NG, NEC, SG]
    eoT_full = np.ascontiguousarray(eo_bf16.transpose(1, 2, 0))
    in_maps = []
    for idx in range(NCORES):
        bsl = slice(idx * BC, (idx + 1) * BC)
        hT_i = np.ascontiguousarray(h[bsl].T.reshape(NKC, P, BC).transpose(1, 0, 2))
        eoT_i = np.ascontiguousarray(
            eoT_full[bsl]
            .reshape(BC, NEC, P, NG, SG)
            .transpose(0, 2, 3, 1, 4)                 # [bb, p, g, c, j]
        )
        in_maps.append(
            {"eoT": eoT_i, "weT": weT, "whT": whT, "hT": hT_i, "bT": bT,
             "vm": vm}
        )

    try:
        res = run_bass_kernel_spmd(nc, in_maps, list(range(NCORES)))
    except Exception:
        # transient NRT/device hiccups happen; one retry
        res = run_bass_kernel_spmd(nc, in_maps, list(range(NCORES)))
    global _last_results
    _last_results = res
    out = np.concatenate([res.results[i]["out"] for i in range(NCORES)], axis=0)
    return out


_last_results = None


if __name__ == "__main__":
    rng = np.random.default_rng(0)
    inputs = {
        "hidden": rng.standard_normal((1, B, D), dtype=np.float32),
        "encoder_outputs": rng.standard_normal((S, B, E2), dtype=np.float32),
        "W": (rng.standard_normal((D, E2 + D)) * 0.02).astype(np.float32),
        "b": (rng.standard_normal((D,)) * 0.02).astype(np.float32),
        "v": rng.random((D,), dtype=np.float32),
    }
    out = kernel(**inputs)
    print("out", out.shape, out.dtype, out.sum())


# revision 16
# speedup vs baseline: 1.0340x; 1.0340x over previous
"""Cross-attention (Bahdanau-style) scores kernel for 8 Trainium2 NeuronCores.

Reference computation (per batch b, source position s):
    energy[b,s,:] = tanh(Wh @ h[b] + We @ eo[s,b] + bias)
    scores[b,s]   = v . energy[b,s,:]
    out[b,:]      = softmax(scores[b,:])   over s

Sharding: data-parallel over batch (64 batches -> 8 per core). Weights are
replicated. No collectives needed (softmax is per-batch, fully local).

Per-core structure (S=4096, Bc=8, E2=512, D=256), blocks of 4 batches:
  - DMA pre-transposed eoT tiles [p=128, c=4, s=512] bf16 (4 KiB lines)
  - PE proj: for kc, c: one We chunk stationary, streamed over 4 batch
    tiles (stationary reuse -> LDWEIGHTS hides under matmuls)
  - ACT: energy = tanh(psum + baseT[k,bb]) fused bias, -> bf16 SBUF
  - PE dot: v . energy via col-tiled [128,32] matmuls, 4 concurrent
    quadrants (tile_position=(0,32q)) -> scores land on partition
    32*(bb%4) + bb//4 of a full [128,512] PSUM bank
  - ACT: exp per s-group with accum -> per-group sums
  - tail: DVE reduce + reciprocal + scale, 8 row DMAs out
"""

import numpy as np
import ml_dtypes

import concourse.bass as bass
import concourse.bacc as bacc
import concourse.tile as tile
from concourse import mybir
from concourse.bass_utils import run_bass_kernel_spmd

dt = mybir.dt

S = 4096          # src_len
B = 64            # global batch
E2 = 512          # 2*enc_hid
D = 256           # dec_hid
NCORES = 8
BC = B // NCORES  # batches per core = 8
P = 128
SG = 512          # s-group size
NG = S // SG      # 8 s-groups
NEC = E2 // P     # 4 e-chunks
NKC = D // P      # 2 k-chunks
HB = 4            # batches per block-half
NB = NG * 2       # 16 blocks (s-group x batch-half)

F32 = dt.float32
BF16 = dt.bfloat16


def _p_of(bb):
    """Partition where batch bb's scores land (quadrant q=bb%4, col bb//4)."""
    return 32 * (bb % 4) + bb // 4


def build_program():
    nc = bacc.Bacc(None, target_bir_lowering=False, debug=False, num_devices=8)

    # eoT[bb, p, g, c, j] = eo[g*512+j, bb, c*128+p]  (host pre-transposed, bf16)
    eoT_d = nc.declare_dram_parameter("eoT", [BC, P, NG, NEC, SG], BF16, isOutput=False)
    # weT[p, c, k] = We.T[c*128+p, k] ; We = W[:, D:]
    weT_d = nc.declare_dram_parameter("weT", [P, NEC, D], BF16, isOutput=False)
    # whT[p, dc, kc, j] = W[kc*128+j, dc*128+p]  (Wh part, pre-chunked)
    whT_d = nc.declare_dram_parameter("whT", [P, NKC, NKC, P], F32, isOutput=False)
    # hT[p, dc, bb] = h[bb, dc*128+p]
    hT_d = nc.declare_dram_parameter("hT", [P, NKC, BC], F32, isOutput=False)
    # bT[p, kc] = bias[kc*128+p]
    bT_d = nc.declare_dram_parameter("bT", [P, NKC], F32, isOutput=False)
    # vm[p, kc, bb, col] = v[kc*128+p] if col == bb//4 else 0
    vm_d = nc.declare_dram_parameter("vm", [P, NKC, BC, 32], BF16, isOutput=False)
    out_d = nc.declare_dram_parameter("out", [BC, S], F32, isOutput=True)

    with tile.TileContext(nc) as tc:
        with tc.tile_pool(name="consts", bufs=1) as consts:
            # DMA priority: weT + block-0 tiles feed the first matmuls;
            # everything else can land later
            weT = consts.tile([P, NEC, D], BF16)
            nc.sync.dma_start(out=weT, in_=weT_d[:])
            whT = consts.tile([P, NKC, NKC, P], F32)
            nc.sync.dma_start(out=whT, in_=whT_d[:])
            hT = consts.tile([P, NKC, BC], F32)
            nc.sync.dma_start(out=hT, in_=hT_d[:])
            bT = consts.tile([P, NKC], F32)
            nc.sync.dma_start(out=bT, in_=bT_d[:])
            eot0 = consts.tile([P, HB, NEC, SG], BF16, name="eot0")
            for i in range(HB):
                for c in range(NEC):
                    nc.sync.dma_start(
                        out=eot0[:, i, c, :], in_=eoT_d[i, :, 0, c, :]
                    )
            vm = consts.tile([P, NKC, BC, 32], BF16)
            nc.sync.dma_start(out=vm, in_=vm_d[:])

            baseT = consts.tile([P, NKC, BC], F32)   # [k128, kc, bb]
            esums = consts.tile([P, NG], F32)        # per-group exp sums
            out_sb = consts.tile([P, S], F32)        # exp(scores), batch on partition

            # --- init: baseT[k, bb] = sum_d Wh[k, d] h[bb, d] + bias[k] ---
            with tc.tile_pool(name="initps", bufs=1, space="PSUM") as initps:
                ps_base = initps.tile([P, NKC, BC], F32)
                for kc in range(NKC):
                    for dc in range(NKC):
                        nc.tensor.matmul(
                            ps_base[:, kc, :],
                            whT[:, dc, kc, :],
                            hT[:, dc, :],
                            start=(dc == 0),
                            stop=(dc == NKC - 1),
                        )
                for kc in range(NKC):
                    nc.vector.tensor_scalar_add(
                        baseT[:, kc, :], ps_base[:, kc, :], bT[:, kc : kc + 1]
                    )

            with (
                tc.tile_pool(name="eot", bufs=12) as eot_pool,
                tc.tile_pool(name="en", bufs=16) as en_pool,
                tc.tile_pool(name="pep", bufs=6, space="PSUM") as pep_pool,
                tc.tile_pool(name="psc", bufs=2, space="PSUM") as psc_pool,
            ):
                pend = None  # (g, half, en tiles of previous block)
                ps_sc = None

                def emit_dots(g, half, ens):
                    """Col-tiled dot matmuls for one block-half; 4 concurrent
                    quadrants. Accumulation group per quadrant spans both
                    halves (bb and bb+4) and both kc."""
                    nonlocal ps_sc
                    if half == 0:
                        ps_sc = psc_pool.tile([P, SG], F32, tag="psc")
                    for kc in range(NKC):
                        for i in range(HB):
                            bb = half * HB + i
                            q = bb % 4
                            nc.tensor.matmul(
                                ps_sc[32 * q : 32 * q + 32, :],
                                vm[:, kc, bb, :],
                                ens[(i, kc)],
                                start=(half == 0 and kc == 0),
                                stop=(half == 1 and kc == NKC - 1),
                                tile_position=(0, 32 * q),
                            )
                    if half == 1:
                        s0 = g * SG
                        nc.scalar.activation(
                            out=out_sb[:, s0 : s0 + SG], in_=ps_sc,
                            func=mybir.ActivationFunctionType.Exp,
                            accum_out=esums[:, g : g + 1],
                        )

                for b in range(NB):
                    g, half = b // 2, b % 2
                    # ---- DMA this block's 4 batch tiles (4 KiB lines) ----
                    if b == 0:
                        # block 0 was DMA'd with the consts (high priority)
                        eots = [eot0[:, i] for i in range(HB)]
                    else:
                        eots = []
                        for i in range(HB):
                            bb = half * HB + i
                            t = eot_pool.tile([P, NEC, SG], BF16, tag="eot")
                            nc.sync.dma_start(out=t, in_=eoT_d[bb, :, g])
                            eots.append(t)

                    ens = {}
                    if b == 0:
                        # first block: batch-outer so the first matmul only
                        # waits for tile 0, streaming as the DMAs land
                        for i in range(HB):
                            for kc in range(NKC):
                                ps = pep_pool.tile(
                                    [P, SG], F32, tag="pep", name=f"ps0_{kc}_{i}"
                                )
                                for c in range(NEC):
                                    nc.tensor.matmul(
                                        ps, weT[:, c, kc * P : (kc + 1) * P],
                                        eots[i][:, c, :],
                                        start=(c == 0), stop=(c == NEC - 1),
                                    )
                                en = en_pool.tile([P, SG], BF16, tag="en",
                                                  name=f"en0_{kc}_{i}")
                                nc.scalar.activation(
                                    out=en, in_=ps,
                                    func=mybir.ActivationFunctionType.Tanh,
                                    bias=baseT[:, kc, i : i + 1],
                                )
                                ens[(i, kc)] = en
                        pend = (g, half, ens)
                        continue
                    for kc in range(NKC):
                        # ---- projection, We chunk stationary reused 4x ----
                        pss = [
                            pep_pool.tile([P, SG], F32, tag="pep", name=f"ps_{b}_{kc}_{i}")
                            for i in range(HB)
                        ]
                        for c in range(NEC):
                            lhs = weT[:, c, kc * P : (kc + 1) * P]
                            for i in range(HB):
                                nc.tensor.matmul(
                                    pss[i], lhs, eots[i][:, c, :],
                                    start=(c == 0), stop=(c == NEC - 1),
                                )
                        # ---- tanh(+base bias) -> bf16 energies ----
                        for i in range(HB):
                            bb = half * HB + i
                            en = en_pool.tile([P, SG], BF16, tag="en")
                            nc.scalar.activation(
                                out=en, in_=pss[i],
                                func=mybir.ActivationFunctionType.Tanh,
                                bias=baseT[:, kc, bb : bb + 1],
                            )
                            ens[(i, kc)] = en
                        # previous block's dots go after this block's first
                        # proj wave so their tanhs have finished
                        if kc == 0 and pend is not None:
                            emit_dots(*pend)
                            pend = None
                    pend = (g, half, ens)

                emit_dots(*pend)

                # ---- softmax tail: scale halves on ACT + DVE in parallel,
                # then many small output DMAs to spread across queues ----
                with tc.tile_pool(name="sm", bufs=1) as sm:
                    esum = sm.tile([P, 1], F32)
                    nc.vector.tensor_reduce(
                        out=esum, in_=esums, axis=mybir.AxisListType.X,
                        op=mybir.AluOpType.add,
                    )
                    rsum = sm.tile([P, 1], F32)
                    nc.vector.reciprocal(rsum, esum)
                    # DVE is ~1.6x faster per column than ACT Copy: split 5:3
                    H = 2560
                    nc.vector.tensor_scalar_mul(
                        out_sb[:, :H], out_sb[:, :H], rsum
                    )
                    nc.scalar.activation(
                        out=out_sb[:, H:], in_=out_sb[:, H:],
                        func=mybir.ActivationFunctionType.Copy,
                        scale=rsum,
                    )
                    # scores for batch bb live on partition 32*(bb%4)+bb//4;
                    # stride-32 partition APs: one 4-row DMA per column group
                    osv = out_sb.rearrange("(q r) s -> q r s", q=4)
                    nc.sync.dma_start(out=out_d[0:4, :], in_=osv[:, 0, :])
                    nc.sync.dma_start(out=out_d[4:8, :], in_=osv[:, 1, :])

    return nc


_nc = None


def _get_nc():
    global _nc
    if _nc is None:
        _nc = build_program()
        _nc.compile()
    return _nc


def kernel(hidden, encoder_outputs, W, b, v):
    hidden = np.asarray(hidden, dtype=np.float32)
    encoder_outputs = np.ascontiguousarray(encoder_outputs, dtype=np.float32)
    W = np.asarray(W, dtype=np.float32)
    b = np.asarray(b, dtype=np.float32)
    v = np.asarray(v, dtype=np.float32)

    # host-side prep of the small replicated weights
    We = W[:, D:]                                     # [256, 512]
    weT = np.ascontiguousarray(
        We.T.reshape(NEC, P, D).transpose(1, 0, 2)    # [p, c, k]
    ).astype(ml_dtypes.bfloat16)
    Wh = W[:, :D]                                     # [k, d]
    whT = np.ascontiguousarray(
        Wh.reshape(NKC, P, NKC, P).transpose(3, 2, 0, 1)  # [p(d), dc, kc, j(k)]
    )
    bT = np.ascontiguousarray(b.reshape(NKC, P).T)    # [p, kc]
    vT = np.ascontiguousarray(v.reshape(NKC, P).T)    # [p, kc]
    vm = np.zeros((P, NKC, BC, 32), dtype=np.float32)
    for bb in range(BC):
        vm[:, :, bb, bb // 4] = vT
    vm = vm.astype(ml_dtypes.bfloat16)
    h = hidden[0]                                     # [64, 256]

    nc = _get_nc()
    eo_bf16 = encoder_outputs.astype(ml_dtypes.bfloat16)
    # [S, B, E2] -> [B, E2, S]; per-core slice repacks to [BC, P, NG, NEC, SG]
    eoT_full = np.ascontiguousarray(eo_bf16.transpose(1, 2, 0))
    in_maps = []
    for idx in range(NCORES):
        bsl = slice(idx * BC, (idx + 1) * BC)
        hT_i = np.ascontiguousarray(h[bsl].T.reshape(NKC, P, BC).transpose(1, 0, 2))
        eoT_i = np.ascontiguousarray(
            eoT_full[bsl]
            .reshape(BC, NEC, P, NG, SG)
            .transpose(0, 2, 3, 1, 4)                 # [bb, p, g, c, j]
        )
        in_maps.append(
            {"eoT": eoT_i, "weT": weT, "whT": whT, "hT": hT_i, "bT": bT,
             "vm": vm}
        )

    try:
        res = run_bass_kernel_spmd(nc, in_maps, list(range(NCORES)))
    except Exception:
        # transient NRT/device hiccups happen; one retry
        res = run_bass_kernel_spmd(nc, in_maps, list(range(NCORES)))
    global _last_results
    _last_results = res
    out = np.concatenate([res.results[i]["out"] for i in range(NCORES)], axis=0)
    return out


_last_results = None


if __name__ == "__main__":
    rng = np.random.default_rng(0)
    inputs = {
        "hidden": rng.standard_normal((1, B, D), dtype=np.float32),
        "encoder_outputs": rng.standard_normal((S, B, E2), dtype=np.float32),
        "W": (rng.standard_normal((D, E2 + D)) * 0.02).astype(np.float32),
        "b": (rng.standard_normal((D,)) * 0.02).astype(np.float32),
        "v": rng.random((D,), dtype=np.float32),
    }
    out = kernel(**inputs)
    print("out", out.shape, out.dtype, out.sum())


# revision 19
# speedup vs baseline: 1.0583x; 1.0234x over previous
"""Cross-attention (Bahdanau-style) scores kernel for 8 Trainium2 NeuronCores.

Reference computation (per batch b, source position s):
    energy[b,s,:] = tanh(Wh @ h[b] + We @ eo[s,b] + bias)
    scores[b,s]   = v . energy[b,s,:]
    out[b,:]      = softmax(scores[b,:])   over s

Sharding: data-parallel over batch (64 batches -> 8 per core). Weights are
replicated. No collectives needed (softmax is per-batch, fully local).

Per-core structure (S=4096, Bc=8, E2=512, D=256), blocks of 4 batches:
  - DMA pre-transposed eoT tiles [p=128, c=4, s=512] bf16 (4 KiB lines)
  - PE proj: for kc, c: one We chunk stationary, streamed over 4 batch
    tiles (stationary reuse -> LDWEIGHTS hides under matmuls)
  - ACT: energy = tanh(psum + baseT[k,bb]) fused bias, -> bf16 SBUF
  - PE dot: v . energy via col-tiled [128,32] matmuls, 4 concurrent
    quadrants (tile_position=(0,32q)) -> scores land on partition
    32*(bb%4) + bb//4 of a full [128,512] PSUM bank
  - ACT: exp per s-group with accum -> per-group sums
  - tail: DVE reduce + reciprocal + scale, 8 row DMAs out
"""

import numpy as np
import ml_dtypes

import concourse.bass as bass
import concourse.bacc as bacc
import concourse.tile as tile
from concourse import mybir
from concourse.bass_utils import run_bass_kernel_spmd

dt = mybir.dt

S = 4096          # src_len
B = 64            # global batch
E2 = 512          # 2*enc_hid
D = 256           # dec_hid
NCORES = 8
BC = B // NCORES  # batches per core = 8
P = 128
SG = 512          # s-group size
NG = S // SG      # 8 s-groups
NEC = E2 // P     # 4 e-chunks
NKC = D // P      # 2 k-chunks
HB = 4            # batches per block-half
NB = NG * 2       # 16 blocks (s-group x batch-half)

F32 = dt.float32
BF16 = dt.bfloat16


def _p_of(bb):
    """Partition where batch bb's scores land (quadrant q=bb%4, col bb//4)."""
    return 32 * (bb % 4) + bb // 4


def build_program():
    nc = bacc.Bacc(None, target_bir_lowering=False, debug=False, num_devices=8)

    # eoT[bb, p, g, c, j] = eo[g*512+j, bb, c*128+p]  (host pre-transposed, bf16)
    eoT_d = nc.declare_dram_parameter("eoT", [BC, P, NG, NEC, SG], BF16, isOutput=False)
    # weT[p, c, k] = We.T[c*128+p, k] ; We = W[:, D:]
    weT_d = nc.declare_dram_parameter("weT", [P, NEC, D], BF16, isOutput=False)
    # whT[p, dc, kc, j] = W[kc*128+j, dc*128+p]  (Wh part, pre-chunked)
    whT_d = nc.declare_dram_parameter("whT", [P, NKC, NKC, P], F32, isOutput=False)
    # hT[p, dc, bb] = h[bb, dc*128+p]
    hT_d = nc.declare_dram_parameter("hT", [P, NKC, BC], F32, isOutput=False)
    # bT[p, kc] = bias[kc*128+p]
    bT_d = nc.declare_dram_parameter("bT", [P, NKC], F32, isOutput=False)
    # vm[p, kc, bb, col] = v[kc*128+p] if col == bb//4 else 0
    vm_d = nc.declare_dram_parameter("vm", [P, NKC, BC, 32], BF16, isOutput=False)
    out_d = nc.declare_dram_parameter("out", [BC, S], F32, isOutput=True)

    with tile.TileContext(nc) as tc:
        with tc.tile_pool(name="consts", bufs=1) as consts:
            # DMA priority: weT + block-0 tiles feed the first matmuls;
            # everything else can land later
            weT = consts.tile([P, NEC, D], BF16)
            nc.sync.dma_start(out=weT, in_=weT_d[:])
            whT = consts.tile([P, NKC, NKC, P], F32)
            nc.sync.dma_start(out=whT, in_=whT_d[:])
            hT = consts.tile([P, NKC, BC], F32)
            nc.sync.dma_start(out=hT, in_=hT_d[:])
            bT = consts.tile([P, NKC], F32)
            nc.sync.dma_start(out=bT, in_=bT_d[:])
            vm = consts.tile([P, NKC, BC, 32], BF16)
            nc.sync.dma_start(out=vm, in_=vm_d[:])

            baseT = consts.tile([P, NKC, BC], F32)   # [k128, kc, bb]
            esums = consts.tile([P, NG], F32)        # per-group exp sums
            out_sb = consts.tile([P, S], F32)        # exp(scores), batch on partition

            # --- init: baseT[k, bb] = sum_d Wh[k, d] h[bb, d] + bias[k] ---
            with tc.tile_pool(name="initps", bufs=1, space="PSUM") as initps:
                ps_base = initps.tile([P, NKC, BC], F32)
                for kc in range(NKC):
                    for dc in range(NKC):
                        nc.tensor.matmul(
                            ps_base[:, kc, :],
                            whT[:, dc, kc, :],
                            hT[:, dc, :],
                            start=(dc == 0),
                            stop=(dc == NKC - 1),
                        )
                for kc in range(NKC):
                    nc.vector.tensor_scalar_add(
                        baseT[:, kc, :], ps_base[:, kc, :], bT[:, kc : kc + 1]
                    )

            with (
                tc.tile_pool(name="eot", bufs=20) as eot_pool,
                tc.tile_pool(name="en", bufs=16) as en_pool,
                tc.tile_pool(name="pep", bufs=6, space="PSUM") as pep_pool,
                tc.tile_pool(name="psc", bufs=2, space="PSUM") as psc_pool,
            ):
                pend = None  # (g, half, en tiles of previous block)
                ps_sc = None

                def emit_dots(g, half, ens):
                    """Col-tiled dot matmuls for one block-half; 4 concurrent
                    quadrants. Accumulation group per quadrant spans both
                    halves (bb and bb+4) and both kc."""
                    nonlocal ps_sc
                    if half == 0:
                        ps_sc = psc_pool.tile([P, SG], F32, tag="psc")
                    for kc in range(NKC):
                        for i in range(HB):
                            bb = half * HB + i
                            q = bb % 4
                            nc.tensor.matmul(
                                ps_sc[32 * q : 32 * q + 32, :],
                                vm[:, kc, bb, :],
                                ens[(i, kc)],
                                start=(half == 0 and kc == 0),
                                stop=(half == 1 and kc == NKC - 1),
                                tile_position=(0, 32 * q),
                            )
                    if half == 1:
                        s0 = g * SG
                        nc.scalar.activation(
                            out=out_sb[:, s0 : s0 + SG], in_=ps_sc,
                            func=mybir.ActivationFunctionType.Exp,
                            accum_out=esums[:, g : g + 1],
                        )

                for b in range(NB):
                    g, half = b // 2, b % 2
                    # ---- DMA this block's 4 batch tiles (4 KiB lines) ----
                    eots = []
                    for i in range(HB):
                        bb = half * HB + i
                        t = eot_pool.tile([P, NEC, SG], BF16, tag="eot")
                        if b == 0:
                            # halve the first block's DMAs so the first
                            # matmul starts as early as possible
                            nc.sync.dma_start(
                                out=t[:, :2, :], in_=eoT_d[bb, :, g, :2, :]
                            )
                            nc.sync.dma_start(
                                out=t[:, 2:, :], in_=eoT_d[bb, :, g, 2:, :]
                            )
                        else:
                            nc.sync.dma_start(out=t, in_=eoT_d[bb, :, g])
                        eots.append(t)

                    ens = {}
                    if b == 0:
                        # first block: batch-outer so the first matmul only
                        # waits for tile 0, streaming as the DMAs land
                        for i in range(HB):
                            for kc in range(NKC):
                                ps = pep_pool.tile(
                                    [P, SG], F32, tag="pep", name=f"ps0_{kc}_{i}"
                                )
                                for c in range(NEC):
                                    nc.tensor.matmul(
                                        ps, weT[:, c, kc * P : (kc + 1) * P],
                                        eots[i][:, c, :],
                                        start=(c == 0), stop=(c == NEC - 1),
                                    )
                                en = en_pool.tile([P, SG], BF16, tag="en",
                                                  name=f"en0_{kc}_{i}")
                                nc.scalar.activation(
                                    out=en, in_=ps,
                                    func=mybir.ActivationFunctionType.Tanh,
                                    bias=baseT[:, kc, i : i + 1],
                                )
                                ens[(i, kc)] = en
                        pend = (g, half, ens)
                        continue
                    for kc in range(NKC):
                        # ---- projection, We chunk stationary reused 4x ----
                        pss = [
                            pep_pool.tile([P, SG], F32, tag="pep", name=f"ps_{b}_{kc}_{i}")
                            for i in range(HB)
                        ]
                        for c in range(NEC):
                            lhs = weT[:, c, kc * P : (kc + 1) * P]
                            for i in range(HB):
                                nc.tensor.matmul(
                                    pss[i], lhs, eots[i][:, c, :],
                                    start=(c == 0), stop=(c == NEC - 1),
                                )
                        # ---- tanh(+base bias) -> bf16 energies ----
                        for i in range(HB):
                            bb = half * HB + i
                            en = en_pool.tile([P, SG], BF16, tag="en")
                            nc.scalar.activation(
                                out=en, in_=pss[i],
                                func=mybir.ActivationFunctionType.Tanh,
                                bias=baseT[:, kc, bb : bb + 1],
                            )
                            ens[(i, kc)] = en
                        # previous block's dots go after this block's first
                        # proj wave so their tanhs have finished
                        if kc == 0 and pend is not None:
                            emit_dots(*pend)
                            pend = None
                    pend = (g, half, ens)

                emit_dots(*pend)

                # ---- softmax tail: scale halves on ACT + DVE in parallel,
                # then many small output DMAs to spread across queues ----
                with tc.tile_pool(name="sm", bufs=1) as sm:
                    esum = sm.tile([P, 1], F32)
                    nc.vector.tensor_reduce(
                        out=esum, in_=esums, axis=mybir.AxisListType.X,
                        op=mybir.AluOpType.add,
                    )
                    rsum = sm.tile([P, 1], F32)
                    nc.vector.reciprocal(rsum, esum)
                    # DVE is ~1.6x faster per column than ACT Copy: split 5:3
                    H = 2560
                    nc.vector.tensor_scalar_mul(
                        out_sb[:, :H], out_sb[:, :H], rsum
                    )
                    nc.scalar.activation(
                        out=out_sb[:, H:], in_=out_sb[:, H:],
                        func=mybir.ActivationFunctionType.Copy,
                        scale=rsum,
                    )
                    # scores for batch bb live on partition 32*(bb%4)+bb//4;
                    # stride-32 partition APs: one 4-row DMA per column group
                    osv = out_sb.rearrange("(q r) s -> q r s", q=4)
                    nc.sync.dma_start(out=out_d[0:4, :], in_=osv[:, 0, :])
                    nc.sync.dma_start(out=out_d[4:8, :], in_=osv[:, 1, :])

    return nc


_nc = None


def _get_nc():
    global _nc
    if _nc is None:
        _nc = build_program()
        _nc.compile()
    return _nc


def kernel(hidden, encoder_outputs, W, b, v):
    hidden = np.asarray(hidden, dtype=np.float32)
    encoder_outputs = np.ascontiguousarray(encoder_outputs, dtype=np.float32)
    W = np.asarray(W, dtype=np.float32)
    b = np.asarray(b, dtype=np.float32)
    v = np.asarray(v, dtype=np.float32)

    # host-side prep of the small replicated weights
    We = W[:, D:]                                     # [256, 512]
    weT = np.ascontiguousarray(
        We.T.reshape(NEC, P, D).transpose(1, 0, 2)    # [p, c, k]
    ).astype(ml_dtypes.bfloat16)
    Wh = W[:, :D]                                     # [k, d]
    whT = np.ascontiguousarray(
        Wh.reshape(NKC, P, NKC, P).transpose(3, 2, 0, 1)  # [p(d), dc, kc, j(k)]
    )
    bT = np.ascontiguousarray(b.reshape(NKC, P).T)    # [p, kc]
    vT = np.ascontiguousarray(v.reshape(NKC, P).T)    # [p, kc]
    vm = np.zeros((P, NKC, BC, 32), dtype=np.float32)
    for bb in range(BC):
        vm[:, :, bb, bb // 4] = vT
    vm = vm.astype(ml_dtypes.bfloat16)
    h = hidden[0]                                     # [64, 256]

    nc = _get_nc()
    eo_bf16 = encoder_outputs.astype(ml_dtypes.bfloat16)
    # [S, B, E2] -> [B, E2, S]; per-core slice repacks to [BC, P, NG, NEC, SG]
    eoT_full = np.ascontiguousarray(eo_bf16.transpose(1, 2, 0))
    in_maps = []
    for idx in range(NCORES):
        bsl = slice(idx * BC, (idx + 1) * BC)
        hT_i = np.ascontiguousarray(h[bsl].T.reshape(NKC, P, BC).transpose(1, 0, 2))
        eoT_i = np.ascontiguousarray(
            eoT_full[bsl]
            .reshape(BC, NEC, P, NG, SG)
            .transpose(0, 2, 3, 1, 4)                 # [bb, p, g, c, j]
        )
        in_maps.append(
            {"eoT": eoT_i, "weT": weT, "whT": whT, "hT": hT_i, "bT": bT,
             "vm": vm}
        )

    try:
        res = run_bass_kernel_spmd(nc, in_maps, list(range(NCORES)))
    except Exception:
        # transient NRT/device hiccups happen; one retry
        res = run_bass_kernel_spmd(nc, in_maps, list(range(NCORES)))
    global _last_results
    _last_results = res
    out = np.concatenate([res.results[i]["out"] for i in range(NCORES)], axis=0)
    return out


_last_results = None


if __name__ == "__main__":
    rng = np.random.default_rng(0)
    inputs = {
        "hidden": rng.standard_normal((1, B, D), dtype=np.float32),
        "encoder_outputs": rng.standard_normal((S, B, E2), dtype=np.float32),
        "W": (rng.standard_normal((D, E2 + D)) * 0.02).astype(np.float32),
        "b": (rng.standard_normal((D,)) * 0.02).astype(np.float32),
        "v": rng.random((D,), dtype=np.float32),
    }
    out = kernel(**inputs)
    print("out", out.shape, out.dtype, out.sum())


# revision 20
# speedup vs baseline: 1.0721x; 1.0130x over previous
"""Cross-attention (Bahdanau-style) scores kernel for 8 Trainium2 NeuronCores.

Reference computation (per batch b, source position s):
    energy[b,s,:] = tanh(Wh @ h[b] + We @ eo[s,b] + bias)
    scores[b,s]   = v . energy[b,s,:]
    out[b,:]      = softmax(scores[b,:])   over s

Sharding: data-parallel over batch (64 batches -> 8 per core). Weights are
replicated. No collectives needed (softmax is per-batch, fully local).

Per-core structure (S=4096, Bc=8, E2=512, D=256), blocks of 4 batches:
  - DMA pre-transposed eoT tiles [p=128, c=4, s=512] bf16 (4 KiB lines)
  - PE proj: for kc, c: one We chunk stationary, streamed over 4 batch
    tiles (stationary reuse -> LDWEIGHTS hides under matmuls)
  - ACT: energy = tanh(psum + baseT[k,bb]) fused bias, -> bf16 SBUF
  - PE dot: v . energy via col-tiled [128,32] matmuls, 4 concurrent
    quadrants (tile_position=(0,32q)) -> scores land on partition
    32*(bb%4) + bb//4 of a full [128,512] PSUM bank
  - ACT: exp per s-group with accum -> per-group sums
  - tail: DVE reduce + reciprocal + scale, 8 row DMAs out
"""

import numpy as np
import ml_dtypes

import concourse.bass as bass
import concourse.bacc as bacc
import concourse.tile as tile
from concourse import mybir
from concourse.bass_utils import run_bass_kernel_spmd

dt = mybir.dt

S = 4096          # src_len
B = 64            # global batch
E2 = 512          # 2*enc_hid
D = 256           # dec_hid
NCORES = 8
BC = B // NCORES  # batches per core = 8
P = 128
SG = 512          # s-group size
NG = S // SG      # 8 s-groups
NEC = E2 // P     # 4 e-chunks
NKC = D // P      # 2 k-chunks
HB = 4            # batches per block-half
NB = NG * 2       # 16 blocks (s-group x batch-half)

F32 = dt.float32
BF16 = dt.bfloat16


def _p_of(bb):
    """Partition where batch bb's scores land (quadrant q=bb%4, col bb//4)."""
    return 32 * (bb % 4) + bb // 4


def build_program():
    nc = bacc.Bacc(None, target_bir_lowering=False, debug=False, num_devices=8)

    # eoT[bb, p, g, c, j] = eo[g*512+j, bb, c*128+p]  (host pre-transposed, bf16)
    eoT_d = nc.declare_dram_parameter("eoT", [BC, P, NG, NEC, SG], BF16, isOutput=False)
    # weT[p, c, k] = We.T[c*128+p, k] ; We = W[:, D:]
    weT_d = nc.declare_dram_parameter("weT", [P, NEC, D], BF16, isOutput=False)
    # whT[p, dc, kc, j] = W[kc*128+j, dc*128+p]  (Wh part, pre-chunked)
    whT_d = nc.declare_dram_parameter("whT", [P, NKC, NKC, P], F32, isOutput=False)
    # hT[p, dc, bb] = h[bb, dc*128+p]
    hT_d = nc.declare_dram_parameter("hT", [P, NKC, BC], F32, isOutput=False)
    # bT[p, kc] = bias[kc*128+p]
    bT_d = nc.declare_dram_parameter("bT", [P, NKC], F32, isOutput=False)
    # vm[p, kc, bb, col] = v[kc*128+p] if col == bb//4 else 0
    vm_d = nc.declare_dram_parameter("vm", [P, NKC, BC, 32], BF16, isOutput=False)
    out_d = nc.declare_dram_parameter("out", [BC, S], F32, isOutput=True)

    with tile.TileContext(nc) as tc:
        with tc.tile_pool(name="consts", bufs=1) as consts:
            # DMA priority: weT + block-0 tiles feed the first matmuls;
            # everything else can land later
            weT = consts.tile([P, NEC, D], BF16)
            nc.sync.dma_start(out=weT, in_=weT_d[:])
            whT = consts.tile([P, NKC, NKC, P], F32)
            nc.sync.dma_start(out=whT, in_=whT_d[:])
            hT = consts.tile([P, NKC, BC], F32)
            nc.sync.dma_start(out=hT, in_=hT_d[:])
            bT = consts.tile([P, NKC], F32)
            nc.sync.dma_start(out=bT, in_=bT_d[:])
            vm = consts.tile([P, NKC, BC, 32], BF16)
            nc.sync.dma_start(out=vm, in_=vm_d[:])

            baseT = consts.tile([P, NKC, BC], F32)   # [k128, kc, bb]
            esums = consts.tile([P, NG], F32)        # per-group exp sums
            out_sb = consts.tile([P, S], F32)        # exp(scores), batch on partition

            # --- init: baseT[k, bb] = sum_d Wh[k, d] h[bb, d] + bias[k] ---
            with tc.tile_pool(name="initps", bufs=1, space="PSUM") as initps:
                ps_base = initps.tile([P, NKC, BC], F32)
                for kc in range(NKC):
                    for dc in range(NKC):
                        nc.tensor.matmul(
                            ps_base[:, kc, :],
                            whT[:, dc, kc, :],
                            hT[:, dc, :],
                            start=(dc == 0),
                            stop=(dc == NKC - 1),
                        )
                for kc in range(NKC):
                    nc.vector.tensor_scalar_add(
                        baseT[:, kc, :], ps_base[:, kc, :], bT[:, kc : kc + 1]
                    )

            with (
                tc.tile_pool(name="eot", bufs=20) as eot_pool,
                tc.tile_pool(name="en", bufs=16) as en_pool,
                tc.tile_pool(name="pep", bufs=6, space="PSUM") as pep_pool,
                tc.tile_pool(name="psc", bufs=2, space="PSUM") as psc_pool,
            ):
                pend = None  # (g, half, en tiles of previous block)
                ps_sc = None

                def emit_dots(g, half, ens):
                    """Col-tiled dot matmuls for one block-half; 4 concurrent
                    quadrants. Accumulation group per quadrant spans both
                    halves (bb and bb+4) and both kc."""
                    nonlocal ps_sc
                    if half == 0:
                        ps_sc = psc_pool.tile([P, SG], F32, tag="psc")
                    for kc in range(NKC):
                        for i in range(HB):
                            bb = half * HB + i
                            q = bb % 4
                            nc.tensor.matmul(
                                ps_sc[32 * q : 32 * q + 32, :],
                                vm[:, kc, bb, :],
                                ens[(i, kc)],
                                start=(half == 0 and kc == 0),
                                stop=(half == 1 and kc == NKC - 1),
                                tile_position=(0, 32 * q),
                            )
                    if half == 1:
                        s0 = g * SG
                        nc.scalar.activation(
                            out=out_sb[:, s0 : s0 + SG], in_=ps_sc,
                            func=mybir.ActivationFunctionType.Exp,
                            accum_out=esums[:, g : g + 1],
                        )

                for b in range(NB):
                    g, half = b // 2, b % 2
                    # ---- DMA this block's 4 batch tiles (4 KiB lines) ----
                    eots = []
                    for i in range(HB):
                        bb = half * HB + i
                        t = eot_pool.tile([P, NEC, SG], BF16, tag="eot")
                        if b == 0:
                            # halve the first block's DMAs so the first
                            # matmul starts as early as possible
                            nc.sync.dma_start(
                                out=t[:, :2, :], in_=eoT_d[bb, :, g, :2, :]
                            )
                            nc.sync.dma_start(
                                out=t[:, 2:, :], in_=eoT_d[bb, :, g, 2:, :]
                            )
                        else:
                            nc.sync.dma_start(out=t, in_=eoT_d[bb, :, g])
                        eots.append(t)

                    ens = {}
                    if b == 0:
                        # first block: batch-outer so the first matmul only
                        # waits for tile 0, streaming as the DMAs land
                        for i in range(HB):
                            for kc in range(NKC):
                                ps = pep_pool.tile(
                                    [P, SG], F32, tag="pep", name=f"ps0_{kc}_{i}"
                                )
                                for c in range(NEC):
                                    nc.tensor.matmul(
                                        ps, weT[:, c, kc * P : (kc + 1) * P],
                                        eots[i][:, c, :],
                                        start=(c == 0), stop=(c == NEC - 1),
                                    )
                                en = en_pool.tile([P, SG], BF16, tag="en",
                                                  name=f"en0_{kc}_{i}")
                                nc.scalar.activation(
                                    out=en, in_=ps,
                                    func=mybir.ActivationFunctionType.Tanh,
                                    bias=baseT[:, kc, i : i + 1],
                                )
                                ens[(i, kc)] = en
                        pend = (g, half, ens)
                        continue
                    for kc in range(NKC):
                        # ---- projection, We chunk stationary reused 4x ----
                        pss = [
                            pep_pool.tile([P, SG], F32, tag="pep", name=f"ps_{b}_{kc}_{i}")
                            for i in range(HB)
                        ]
                        for c in range(NEC):
                            lhs = weT[:, c, kc * P : (kc + 1) * P]
                            for i in range(HB):
                                nc.tensor.matmul(
                                    pss[i], lhs, eots[i][:, c, :],
                                    start=(c == 0), stop=(c == NEC - 1),
                                )
                        # ---- tanh(+base bias) -> bf16 energies ----
                        for i in range(HB):
                            bb = half * HB + i
                            en = en_pool.tile([P, SG], BF16, tag="en")
                            nc.scalar.activation(
                                out=en, in_=pss[i],
                                func=mybir.ActivationFunctionType.Tanh,
                                bias=baseT[:, kc, bb : bb + 1],
                            )
                            ens[(i, kc)] = en
                        # previous block's dots go after this block's first
                        # proj wave so their tanhs have finished
                        if kc == 0 and pend is not None:
                            emit_dots(*pend)
                            pend = None
                    pend = (g, half, ens)

                emit_dots(*pend)

                # ---- softmax tail: scale halves on ACT + DVE in parallel,
                # then many small output DMAs to spread across queues ----
                with tc.tile_pool(name="sm", bufs=1) as sm:
                    esum = sm.tile([P, 1], F32)
                    nc.vector.tensor_reduce(
                        out=esum, in_=esums, axis=mybir.AxisListType.X,
                        op=mybir.AluOpType.add,
                    )
                    rsum = sm.tile([P, 1], F32)
                    nc.vector.reciprocal(rsum, esum)
                    # DVE is ~1.6x faster per column than ACT Copy: split 5:3
                    H = 2560
                    nc.vector.tensor_scalar_mul(
                        out_sb[:, :H], out_sb[:, :H], rsum
                    )
                    nc.scalar.activation(
                        out=out_sb[:, H:], in_=out_sb[:, H:],
                        func=mybir.ActivationFunctionType.Copy,
                        scale=rsum,
                    )
                    # scores for batch bb live on partition 32*(bb%4)+bb//4;
                    # stride-32 partition APs: 4-row DMAs per column group.
                    # Launch DVE-scaled halves from Sync's DGE and ACT-scaled
                    # halves from the Activation DGE so launches parallelize.
                    osv = out_sb.rearrange("(q r) s -> q r s", q=4)
                    nc.sync.dma_start(out=out_d[0:4, :H], in_=osv[:, 0, :H])
                    nc.sync.dma_start(out=out_d[4:8, :H], in_=osv[:, 1, :H])
                    nc.scalar.dma_start(out=out_d[0:4, H:], in_=osv[:, 0, H:])
                    nc.scalar.dma_start(out=out_d[4:8, H:], in_=osv[:, 1, H:])

    return nc


_nc = None


def _get_nc():
    global _nc
    if _nc is None:
        _nc = build_program()
        _nc.compile()
    return _nc


def kernel(hidden, encoder_outputs, W, b, v):
    hidden = np.asarray(hidden, dtype=np.float32)
    encoder_outputs = np.ascontiguousarray(encoder_outputs, dtype=np.float32)
    W = np.asarray(W, dtype=np.float32)
    b = np.asarray(b, dtype=np.float32)
    v = np.asarray(v, dtype=np.float32)

    # host-side prep of the small replicated weights
    We = W[:, D:]                                     # [256, 512]
    weT = np.ascontiguousarray(
        We.T.reshape(NEC, P, D).transpose(1, 0, 2)    # [p, c, k]
    ).astype(ml_dtypes.bfloat16)
    Wh = W[:, :D]                                     # [k, d]
    whT = np.ascontiguousarray(
        Wh.reshape(NKC, P, NKC, P).transpose(3, 2, 0, 1)  # [p(d), dc, kc, j(k)]
    )
    bT = np.ascontiguousarray(b.reshape(NKC, P).T)    # [p, kc]
    vT = np.ascontiguousarray(v.reshape(NKC, P).T)    # [p, kc]
    vm = np.zeros((P, NKC, BC, 32), dtype=np.float32)
    for bb in range(BC):
        vm[:, :, bb, bb // 4] = vT
    vm = vm.astype(ml_dtypes.bfloat16)
    h = hidden[0]                                     # [64, 256]

    nc = _get_nc()
    eo_bf16 = encoder_outputs.astype(ml_dtypes.bfloat16)
    # [S, B, E2] -> [B, E2, S]; per-core slice repacks to [BC, P, NG, NEC, SG]
    eoT_full = np.ascontiguousarray(eo_bf16.transpose(1, 2, 0))
    in_maps = []
    for idx in range(NCORES):
        bsl = slice(idx * BC, (idx + 1) * BC)
        hT_i = np.ascontiguousarray(h[bsl].T.reshape(NKC, P, BC).transpose(1, 0, 2))
        eoT_i = np.ascontiguousarray(
            eoT_full[bsl]
            .reshape(BC, NEC, P, NG, SG)
            .transpose(0, 2, 3, 1, 4)                 # [bb, p, g, c, j]
        )
        in_maps.append(
            {"eoT": eoT_i, "weT": weT, "whT": whT, "hT": hT_i, "bT": bT,
             "vm": vm}
        )

    try:
        res = run_bass_kernel_spmd(nc, in_maps, list(range(NCORES)))
    except Exception:
        # transient NRT/device hiccups happen; one retry
        res = run_bass_kernel_spmd(nc, in_maps, list(range(NCORES)))
    global _last_results
    _last_results = res
    out = np.concatenate([res.results[i]["out"] for i in range(NCORES)], axis=0)
    return out


_last_results = None


if __name__ == "__main__":
    rng = np.random.default_rng(0)
    inputs = {
        "hidden": rng.standard_normal((1, B, D), dtype=np.float32),
        "encoder_outputs": rng.standard_normal((S, B, E2), dtype=np.float32),
        "W": (rng.standard_normal((D, E2 + D)) * 0.02).astype(np.float32),
        "b": (rng.standard_normal((D,)) * 0.02).astype(np.float32),
        "v": rng.random((D,), dtype=np.float32),
    }
    out = kernel(**inputs)
    print("out", out.shape, out.dtype, out.sum())
